# revision 3
# baseline (speedup 1.0000x reference)
"""Trainium2 Bass kernel for nn_DI_GNN_15109694947989 (gnn_message_passing).

Self-contained: kernel(**inputs) takes the FULL inputs, shards the B=128
graphs data-parallel across 8 NeuronCores (16 graphs each), runs one SPMD
Bass/Tile program per core, and gathers the full (affinity, pose) output.

Per-graph device pipeline (N=512 nodes, K=6 kNN, 3 GCN layers, attention
pooling, two heads):
  - every node has exactly K=6 in-edges, so GCN degree == 7 uniformly and
    the aggregation is (A + I) @ hw / 7 with a 0/1 mask; BN (eval mode) and
    the 1/7 fold into the weights on the host.
  - kNN via the DVE top-8 instruction on e' = p_i.p_j - sq_j/2 (monotone in
    -d2; self is always rank 0, so ranks 0..6 = self + 6 nearest and the
    rank-6 value is the threshold).
  - e' is computed on the PE in fp32r with hi/lo splits of pos and sq
    (fp32-accurate at full PE rate); the transposed-orientation matmul uses
    identical products in identical order, so thresholding is exact; the
    -thr subtraction is folded in as two extra contraction rows.
  - layers alternate aggregation (node-major -> feature-major) and feature
    transform (feature-major -> node-major); layer 3 emits feature-major
    for the gate/pooling/head matmuls.
"""
import sys

if "/opt/trn_rl_repo" not in sys.path:
    sys.path.insert(0, "/opt/trn_rl_repo")

from contextlib import ExitStack
import numpy as np

import concourse.bacc as bacc
import concourse.tile as tile
import concourse.mybir as mybir
from concourse._compat import with_exitstack
from concourse.alu_op_type import AluOpType
from concourse.bass_utils import run_bass_kernel_spmd

F32 = mybir.dt.float32
F32R = mybir.dt.float32r
AF = mybir.ActivationFunctionType

B, N, K, IN_FEAT, POS_DIM, H = 128, 512, 6, 128, 3, 256
N_CORES = 8
NG = B // N_CORES          # graphs per core
EPS_BN = 1e-5


def round11(x):
    """Round-to-nearest at 11 mantissa bits == TRN2 fp32r operand rounding."""
    ix = np.ascontiguousarray(x, np.float32).view(np.int32).astype(np.int64)
    return (np.round(ix / 4096.0) * 4096).astype(np.int64).astype(np.int32).view(np.float32)


def host_prep(inp, n_cores, ng):
    f32 = np.float32

    def fold(W, b, g, be, m, v):
        s = g / np.sqrt(v + EPS_BN)
        return (W * s[None, :]).astype(f32), ((b - m) * s + be).astype(f32)

    W1p, b1p = fold(inp["W1"], inp["b1"], inp["g1"], inp["be1"], inp["m1"], inp["v1"])
    W2p, b2p = fold(inp["W2"], inp["b2"], inp["g2"], inp["be2"], inp["m2"], inp["v2"])
    W3p, b3p = fold(inp["W3"], inp["b3"], inp["g3"], inp["be3"], inp["m3"], inp["v3"])
    inv7 = f32(1.0 / 7.0)
    W1f, W2f, W3f = round11(W1p * inv7), round11(W2p * inv7), round11(W3p * inv7)

    def hl(v):
        h = round11(v)
        return h, round11((v - h).astype(f32))

    b1h, b1l = hl(b1p)
    b2h, b2l = hl(b2p)

    consts = {
        "I128": np.eye(128, dtype=f32),
        "ones_row": np.ones((1, 512), f32),
        "ones2": np.ones((2, 512), f32),
        "W1f_x": W1f[:IN_FEAT].copy(),
        "W1f_pos": W1f[IN_FEAT:].copy(),
        "b1_hl": np.stack([b1h, b1l], 0),
        "W2f_0": W2f[:128].copy(),
        "W2f_1": W2f[128:].copy(),
        "b2_hl": np.stack([b2h, b2l], 0),
        "W3f_pack": np.concatenate([W3f[:128, :], W3f[128:, :]], 1),
        "b3cols": np.stack([b3p[:128], b3p[128:]], 1),
        "gate_w": round11(np.stack([inp["gate_w"][:128, 0], inp["gate_w"][128:, 0]], 1)),
        "gate_bc": inp["gate_b"].astype(f32)[None, :],
        "aff_w": round11(np.stack([inp["aff_w"][:128, 0], inp["aff_w"][128:, 0]], 1)),
        "aff_b": inp["aff_b"].astype(f32)[None, :],
        "rl_w1": round11(np.concatenate(
            [inp["rl_w1"][:128, :128], inp["rl_w1"][:128, 128:],
             inp["rl_w1"][128:, :128], inp["rl_w1"][128:, 128:]], 1)),
        "rl_b1": np.stack([inp["rl_b1"][:128], inp["rl_b1"][128:]], 1),
        "rl_w2": round11(np.concatenate([inp["rl_w2"][:128], inp["rl_w2"][128:]], 1)),
        "rl_b2": inp["rl_b2"].astype(f32)[:, None],
    }

    xr = round11(inp["x"])
    pos = inp["pos"]
    posr = round11(pos)

    shards = []
    for c in range(n_cores):
        g0 = c * ng
        sl = slice(g0 * N, (g0 + ng) * N)
        pg_all = pos[sl].reshape(ng, N, POS_DIM)
        eAB = np.zeros((ng, 2, 13, 512), np.float32)
        for g in range(ng):
            pg = pg_all[g]
            ph = round11(pg)
            pl = round11((pg - ph).astype(np.float32))
            sqh = (pg.astype(np.float64) ** 2).sum(1) / 2.0
            sh = round11(sqh)
            slo = round11((sqh - sh).astype(np.float32))
            eA = eAB[g, 0]
            eB = eAB[g, 1]
            eA[0] = 1.0; eA[1] = 1.0
            eA[2:5] = ph.T; eA[5:8] = ph.T; eA[8:11] = pl.T
            eB[0] = -sh; eB[1] = -slo
            eB[2:5] = ph.T; eB[5:8] = pl.T; eB[8:11] = ph.T
            eB[11] = 1.0; eB[12] = 1.0
        shards.append({
            "xr": xr[sl].copy(),
            "posr": posr[sl].copy(),
            "eAB": eAB,
        })
    return consts, shards


@with_exitstack
def build_kernel(ctx: ExitStack, tc: tile.TileContext, ng: int):
    nc = tc.nc

    def dram_in(name, shape):
        return nc.dram_tensor(name, shape, F32, kind="ExternalInput").ap()

    xr = dram_in("xr", [ng * N, IN_FEAT])
    posr = dram_in("posr", [ng * N, POS_DIM])
    eAB = dram_in("eAB", [ng, 2, 13, 512])
    cshapes = {
        "I128": [128, 128], "ones_row": [1, 512], "ones2": [2, 512],
        "W1f_x": [128, 256], "W1f_pos": [3, 256], "b1_hl": [2, 256],
        "W2f_0": [128, 256], "W2f_1": [128, 256], "b2_hl": [2, 256],
        "W3f_pack": [128, 512], "b3cols": [128, 2],
        "gate_w": [128, 2], "gate_bc": [1, 1], "aff_w": [128, 2],
        "aff_b": [1, 1], "rl_w1": [128, 512], "rl_b1": [128, 2],
        "rl_w2": [128, 6], "rl_b2": [3, 1],
    }
    cin = {k: dram_in(k, v) for k, v in cshapes.items()}
    aff_out = nc.dram_tensor("aff_out", [1, ng], F32, kind="ExternalOutput").ap()
    pose_out = nc.dram_tensor("pose_out", [3, ng], F32, kind="ExternalOutput").ap()

    const = ctx.enter_context(tc.tile_pool(name="const", bufs=1))
    work = ctx.enter_context(tc.tile_pool(name="work", bufs=2))
    h4pool = ctx.enter_context(tc.tile_pool(name="h4", bufs=1))
    pool_ps = ctx.enter_context(tc.tile_pool(name="ps", bufs=4, space="PSUM"))
    pool_pss = ctx.enter_context(tc.tile_pool(name="pss", bufs=2, space="PSUM"))

    rdty = {"gate_bc": F32, "aff_b": F32, "rl_b1": F32, "rl_b2": F32,
            "b3cols": F32, "I128": F32}
    csb = {}
    for k, shp in cshapes.items():
        dt = rdty.get(k, F32R)
        t = const.tile(shp, dt, tag=f"c_{k}")
        src = cin[k][:]
        nc.sync.dma_start(t[:], src.bitcast(dt) if dt == F32R else src)
        csb[k] = t

    def rr(ap):
        return ap.bitcast(F32R)

    onesr = csb["ones_row"]
    zrow = const.tile([1, ng], F32, tag="zrow")
    expg_all = []
    gembu0 = const.tile([128, ng], F32, tag="gembu0")
    gembu1 = const.tile([128, ng], F32, tag="gembu1")
    gembu = [gembu0, gembu1]
    h4T_all = []

    for g in range(ng):
        # ---- loads
        eA = work.tile([13, 512], F32R, tag="eA")
        eB = work.tile([13, 512], F32R, tag="eB")
        nc.sync.dma_start(eA[0:11, :], rr(eAB[g, 0, 0:11, :]))
        nc.sync.dma_start(eB[:], rr(eAB[g, 1, :, :]))
        x_sb = []
        pos_sb = []
        for t in range(4):
            xt = work.tile([128, IN_FEAT], F32R, tag=f"x{t}")
            nc.sync.dma_start(xt[:], rr(xr[g * N + t * 128: g * N + (t + 1) * 128, :]))
            x_sb.append(xt)
            pt = work.tile([128, POS_DIM], F32R, tag=f"pos{t}")
            nc.sync.dma_start(pt[:], rr(posr[g * N + t * 128: g * N + (t + 1) * 128, :]))
            pos_sb.append(pt)

        # ---- e (i-orientation) -> InstMax top-8 -> -thr hi/lo rows
        max8 = work.tile([128, 32], F32, tag="max8")
        for t in range(4):
            ep = pool_ps.tile([128, 512], F32, tag="pb")
            nc.tensor.matmul(ep[:], eA[0:11, t * 128:(t + 1) * 128], eB[0:11, :],
                             start=True, stop=True)
            esb = work.tile([128, 512], F32, tag=f"esb{t % 2}")
            nc.scalar.copy(esb[:], ep[:])
            nc.vector.max(out=max8[:, t * 8:(t + 1) * 8], in_=esb[:])

        y = work.tile([1, 512], F32, tag="y")
        for t in range(4):
            thrp = pool_pss.tile([1, 128], F32, tag="psm")
            nc.tensor.transpose(thrp[:], max8[:, 8 * t + 6:8 * t + 7], csb["I128"][:])
            nc.vector.tensor_scalar_mul(y[0:1, t * 128:(t + 1) * 128],
                                        thrp[0:1, :], -1.0)
        nh = work.tile([1, 512], F32R, tag="nh")
        nc.vector.tensor_copy(nh[:], y[:])
        nl = work.tile([1, 512], F32R, tag="nl")
        nc.vector.tensor_tensor(out=nl[:], in0=y[:], in1=nh[:].bitcast(F32),
                                op=AluOpType.subtract)
        nc.sync.dma_start(eA[11:12, :], nh[:])
        nc.sync.dma_start(eA[12:13, :], nl[:])

        # ---- eT (identical products, + folded -thr) -> mask
        maskT = []
        for t in range(4):
            etp = pool_ps.tile([128, 512], F32, tag="pb")
            nc.tensor.matmul(etp[:], eB[:, t * 128:(t + 1) * 128], eA[:],
                             start=True, stop=True)
            mt = work.tile([128, 512], F32R, tag=f"mk{t}")
            nc.vector.tensor_scalar(out=mt[:], in0=etp[:], scalar1=0.0,
                                    scalar2=None, op0=AluOpType.is_ge)
            maskT.append(mt)

        # ---- L1: agg (node -> featT), transform (featT -> node) + relu
        axp = pool_ps.tile([128, 512], F32, tag="pb")
        for j in range(4):
            nc.tensor.matmul(axp[:], x_sb[j][:], maskT[j][:],
                             start=(j == 0), stop=(j == 3))
        app = pool_pss.tile([3, 512], F32, tag="psm")
        for j in range(4):
            nc.tensor.matmul(app[:], pos_sb[j][:], maskT[j][:],
                             start=(j == 0), stop=(j == 3))
        ah1x = work.tile([128, 512], F32R, tag="ah1x")
        nc.scalar.copy(ah1x[:], axp[:])
        ah1p = work.tile([3, 512], F32R, tag="ah1p")
        nc.scalar.copy(ah1p[:], app[:])

        h2 = []
        for i in range(4):
            zp = pool_ps.tile([128, H], F32, tag="pb")
            sl_ = slice(i * 128, (i + 1) * 128)
            nc.tensor.matmul(zp[:], ah1x[:, sl_], csb["W1f_x"][:], start=True, stop=False)
            nc.tensor.matmul(zp[:], ah1p[:, sl_], csb["W1f_pos"][:], start=False, stop=False)
            nc.tensor.matmul(zp[:], csb["ones2"][:, sl_], csb["b1_hl"][:], start=False, stop=True)
            ht = work.tile([128, H], F32R, tag=f"h2_{i}")
            nc.scalar.activation(ht[:], zp[:], AF.Relu)
            h2.append(ht)

        # ---- L2
        ah2 = []
        for m in range(2):
            ap_ = pool_ps.tile([128, 512], F32, tag="pb")
            for j in range(4):
                nc.tensor.matmul(ap_[:], h2[j][:, m * 128:(m + 1) * 128], maskT[j][:],
                                 start=(j == 0), stop=(j == 3))
            aht = work.tile([128, 512], F32R, tag=f"ah2_{m}")
            nc.scalar.copy(aht[:], ap_[:])
            ah2.append(aht)
        h3 = []
        for i in range(4):
            zp = pool_ps.tile([128, H], F32, tag="pb")
            sl_ = slice(i * 128, (i + 1) * 128)
            nc.tensor.matmul(zp[:], ah2[0][:, sl_], csb["W2f_0"][:], start=True, stop=False)
            nc.tensor.matmul(zp[:], ah2[1][:, sl_], csb["W2f_1"][:], start=False, stop=False)
            nc.tensor.matmul(zp[:], csb["ones2"][:, sl_], csb["b2_hl"][:], start=False, stop=True)
            ht = work.tile([128, H], F32R, tag=f"h3_{i}")
            nc.scalar.activation(ht[:], zp[:], AF.Relu)
            h3.append(ht)

        # ---- L3: agg, then featT transform + bias + relu
        ah3 = []
        for m in range(2):
            ap_ = pool_ps.tile([128, 512], F32, tag="pb")
            for j in range(4):
                nc.tensor.matmul(ap_[:], h3[j][:, m * 128:(m + 1) * 128], maskT[j][:],
                                 start=(j == 0), stop=(j == 3))
            aht = work.tile([128, 512], F32R, tag=f"ah3_{m}")
            nc.scalar.copy(aht[:], ap_[:])
            ah3.append(aht)
        h4T = []
        for m in range(2):
            zp = pool_ps.tile([128, 512], F32, tag="pb")
            nc.tensor.matmul(zp[:], csb["W3f_pack"][:, m * 128:(m + 1) * 128],
                             ah3[0][:], start=True, stop=False)
            nc.tensor.matmul(zp[:], csb["W3f_pack"][:, 256 + m * 128:256 + (m + 1) * 128],
                             ah3[1][:], start=False, stop=True)
            ht = h4pool.tile([128, 512], F32R, tag=f"h4T_{g}_{m}")
            nc.scalar.activation(ht[:], zp[:], AF.Relu, bias=csb["b3cols"][:, m:m + 1])
            h4T.append(ht)
        h4T_all.append(h4T)

        # ---- gate -> relu -> exp (+Z) ; Z into zrow[0, g]
        gp = pool_pss.tile([1, 512], F32, tag="psm")
        nc.tensor.matmul(gp[:], csb["gate_w"][:, 0:1], h4T[0][:],
                         start=True, stop=False)
        nc.tensor.matmul(gp[:], csb["gate_w"][:, 1:2], h4T[1][:],
                         start=False, stop=True)
        grow = work.tile([1, 512], F32, tag="grow")
        nc.scalar.activation(grow[:], gp[:], AF.Relu, bias=csb["gate_bc"][0:1, 0:1])
        expg = h4pool.tile([1, 512], F32R, tag=f"expg_{g}")
        zg = work.tile([1, 1], F32, tag="zg")
        nc.scalar.activation(expg[:], grow[:], AF.Exp, accum_out=zg[:])
        nc.sync.dma_start(zrow[0:1, g:g + 1], zg[:])
        expg_all.append(expg)

    # ---- pooling: gemb accumulation + 1/Z normalization
    junk = const.tile([128, 512], F32, tag="junk")
    for g in range(ng):
        expb = pool_ps.tile([128, 512], F32, tag="pb")
        nc.tensor.matmul(expb[:], onesr[0:1, 0:128], expg_all[g][:],
                         start=True, stop=True)
        for m in range(2):
            nc.vector.tensor_tensor(out=junk[:], in0=h4T_all[g][m][:].bitcast(F32),
                                    in1=expb[:], op=AluOpType.mult)
            nc.vector.tensor_reduce(out=gembu[m][:, g:g + 1], in_=junk[:],
                                    axis=mybir.AxisListType.X, op=AluOpType.add)

    rzrow = const.tile([1, ng], F32R, tag="rzrow")
    with nc.allow_low_precision(reason="fp32r rounding of 1/Z is intended"):
        nc.vector.reciprocal(rzrow[:], zrow[:])
    rzb = pool_pss.tile([128, ng], F32, tag="psm")
    nc.tensor.matmul(rzb[:], onesr[0:1, 0:128], rzrow[:], start=True, stop=True)

    gembn = []
    for m in range(2):
        t = const.tile([128, ng], F32R, tag=f"gembn{m}")
        nc.vector.tensor_tensor(out=t[:], in0=gembu[m][:], in1=rzb[:],
                                op=AluOpType.mult)
        gembn.append(t)

    # ---- heads (batched over the core's graphs)
    affp = pool_pss.tile([1, ng], F32, tag="psm")
    nc.tensor.matmul(affp[:], csb["aff_w"][:, 0:1], gembn[0][:], start=True, stop=False)
    nc.tensor.matmul(affp[:], csb["aff_w"][:, 1:2], gembn[1][:], start=False, stop=True)
    affsb = const.tile([1, ng], F32, tag="affsb")
    nc.scalar.activation(affsb[:], affp[:], AF.Identity, bias=csb["aff_b"][0:1, 0:1])
    nc.sync.dma_start(aff_out[:], affsb[:])

    hid = []
    for m in range(2):
        hp = pool_pss.tile([128, ng], F32, tag="psm")
        nc.tensor.matmul(hp[:], csb["rl_w1"][:, m * 128:(m + 1) * 128],
                         gembn[0][:], start=True, stop=False)
        nc.tensor.matmul(hp[:], csb["rl_w1"][:, 256 + m * 128:256 + (m + 1) * 128],
                         gembn[1][:], start=False, stop=True)
        ht = const.tile([128, ng], F32R, tag=f"hid{m}")
        nc.scalar.activation(ht[:], hp[:], AF.Relu, bias=csb["rl_b1"][:, m:m + 1])
        hid.append(ht)

    posep = pool_pss.tile([3, ng], F32, tag="psm")
    nc.tensor.matmul(posep[:], csb["rl_w2"][:, 0:3], hid[0][:], start=True, stop=False)
    nc.tensor.matmul(posep[:], csb["rl_w2"][:, 3:6], hid[1][:], start=False, stop=True)
    posesb = const.tile([3, ng], F32, tag="posesb")
    nc.scalar.activation(posesb[:], posep[:], AF.Identity, bias=csb["rl_b2"][:])
    nc.sync.dma_start(pose_out[:], posesb[:])


_cache = {}


def _get_compiled(ng):
    if ng not in _cache:
        nc = bacc.Bacc("TRN2", target_bir_lowering=False, debug=False,
                       enable_asserts=True, num_devices=N_CORES)
        with tile.TileContext(nc) as tc:
            build_kernel(tc, ng)
        nc.compile()
        _cache[ng] = nc
    return _cache[ng]


def _run(inputs, trace=False, **kw):
    inputs = {k: np.asarray(v) for k, v in inputs.items()}
    consts, shards = host_prep(inputs, N_CORES, NG)
    nc = _get_compiled(NG)
    in_maps = []
    for c in range(N_CORES):
        m = dict(consts)
        m.update(shards[c])
        in_maps.append(m)
    res = run_bass_kernel_spmd(nc, in_maps, core_ids=list(range(N_CORES)),
                               trace=trace, **kw)
    aff = np.concatenate([res.results[c]["aff_out"][0] for c in range(N_CORES)])
    pose = np.concatenate([res.results[c]["pose_out"].T for c in range(N_CORES)], 0)
    return (aff[:, None].astype(np.float32), pose.astype(np.float32)), res


def kernel(**inputs):
    out, _ = _run(inputs, trace=False)
    return out
